# revision 14
# baseline (speedup 1.0000x reference)
"""GQA MultiHeadAttention (B=2, T=2048, D=2048, H=16, KVH=4, HD=128) on 8 trn2 cores.

Sharding: core c -> (batch b = c//4, kv-head g = c%4). Each core computes its
4 query heads' attention + out-projection partial; host sums the 4 partials
per batch (row-parallel Wo all-reduce done on host at unshard time).

Device layout trick: everything stays transposed ([hd, t]) through attention:
  QT/KT = (Wq'^T x^T) with RoPE pair-deinterleave baked into the weight column
  order, S^T = KT-block-as-lhsT @ QT, P = exp(S^T) masked by affine_select,
  ctxT accumulates V-as-lhsT @ P in PSUM, denominator = all-ones matmul of the
  running P-sum (replicated across partitions) -> fast reciprocal -> multiply
  fused with the ctx PSUM eviction. ctxT feeds the out-projection as lhsT
  directly. Matmuls run in float32r (1 cyc/row at N=512 vs 4 for fp32).

Perf notes (from NTFF traces): PE in-order queue means emission order is the
schedule; A(j+1) is emitted before C(j) so the exp/denominator chains of B(j)
hide under A's dense matmuls. Denominator accumulation is split between DVE
and GPSIMD. DMAs are batched via host-side k-tile-major layouts.
"""

import sys

sys.path.insert(0, "/opt/trn_rl_repo")

import numpy as np

B, T, D = 2, 2048, 2048
H, KVH, HD = 16, 4, 128
G = H // KVH  # query heads per kv head (= heads per core)
KB = D // 128  # 16 contraction k-tiles
NJ = T // 512  # 4 t-blocks of 512
MW = (G + 2) * HD  # 768 qkv projection width per core
SCALE = 1.0 / np.sqrt(HD)
N_CORES = 8

_CACHE = {}


def _build():
    import concourse.bass as bass  # noqa: F401
    import concourse.tile as tile
    from concourse import bacc, mybir

    f32 = mybir.dt.float32
    f32r = mybir.dt.float32r
    EXP = mybir.ActivationFunctionType.Exp

    nc = bacc.Bacc(None, target_bir_lowering=False, debug=False)
    xT_d = nc.dram_tensor("xT", [128, KB, T], f32r, kind="ExternalInput")
    wqkv_d = nc.dram_tensor("wqkv", [128, KB, MW], f32r, kind="ExternalInput")
    wo_d = nc.dram_tensor("wo", [128, G, D], f32r, kind="ExternalInput")
    cs_d = nc.dram_tensor("cs", [128, T], f32, kind="ExternalInput")
    consts_d = nc.dram_tensor("consts", [128, 256], f32r, kind="ExternalInput")
    y_d = nc.dram_tensor("y", [T, D], f32, kind="ExternalOutput")

    with tile.TileContext(nc) as tc:
        with (
            tc.tile_pool(name="const", bufs=1) as const,
            tc.tile_pool(name="wq", bufs=1) as wqp,
            tc.tile_pool(name="wop", bufs=1) as wop,
            tc.tile_pool(name="xt", bufs=6) as xtp,
            tc.tile_pool(name="big", bufs=1) as big,
            tc.tile_pool(name="work", bufs=2) as work,
            tc.tile_pool(name="pp", bufs=4) as ppool,
            tc.tile_pool(name="rope", bufs=2) as rope,
            tc.tile_pool(name="yout", bufs=2) as youtp,
            tc.tile_pool(name="ps2", bufs=2, space="PSUM") as ps2,
            tc.tile_pool(name="pss", bufs=2, space="PSUM") as pss,
            tc.tile_pool(name="psc", bufs=2, space="PSUM") as psc,
            tc.tile_pool(name="psy", bufs=2, space="PSUM") as psy,
        ):
            # ---- resident weights / constants ----
            # startup: wqkv on the scalar HWDGE queue, xt(0) on sync, the
            # non-critical loads on gpsimd SWDGE -> three parallel streams
            wqkv_sb = wqp.tile([128, KB, MW], f32r, tag="wqkv")
            xt_tiles = {}

            def load_xt(j, eng=None):
                # chunks of k-tiles for t-block j (finer first chunks on j=0)
                chunks = [4, 4, 4, 4]
                tiles, k0 = [], 0
                for ci, w in enumerate(chunks):
                    t_ = xtp.tile([128, w, 512], f32r, tag="xt", name=f"xt{j}_{ci}",
                                  padded_shape=[128, 4, 512])
                    (eng or nc.sync).dma_start(
                        out=t_[:],
                        in_=xT_d[:, k0 : k0 + w, j * 512 : (j + 1) * 512],
                    )
                    for _ in range(w):
                        tiles.append((t_, k0))
                    k0 += w
                xt_tiles[j] = tiles

            consts_sb = const.tile([128, 256], f32r, tag="consts")
            nc.gpsimd.dma_start(out=consts_sb[:], in_=consts_d[:, :])
            ident = consts_sb[:, 0:128]
            ones = consts_sb[:, 128:256]
            def load_wq_chunk(k0, w):
                nc.sync.dma_start(
                    out=wqkv_sb[:, k0 : k0 + w, :], in_=wqkv_d[:, k0 : k0 + w, :]
                )

            def load_xt_chunk(j, tiles, ci, k0, w):
                t_ = xtp.tile([128, w, 512], f32r, tag="xt", name=f"xt{j}_{ci}",
                              padded_shape=[128, 4, 512])
                nc.sync.dma_start(
                    out=t_[:], in_=xT_d[:, k0 : k0 + w, j * 512 : (j + 1) * 512]
                )
                for _ in range(w):
                    tiles.append((t_, k0))

            xt0_tiles = []
            for ci in range(4):
                load_wq_chunk(4 * ci, 4)
                load_xt_chunk(0, xt0_tiles, ci, 4 * ci, 4)
            xt_tiles[0] = xt0_tiles
            cs_sb = const.tile([128, T], f32, tag="cs")
            nc.sync.dma_start(out=cs_sb[:], in_=cs_d[:, :])
            wo_sb = wop.tile([128, G, D], f32r, tag="wo")
            nc.sync.dma_start(out=wo_sb[:, 0:2, :], in_=wo_d[:, 0:2, :])
            nc.sync.dma_start(out=wo_sb[:, 2:4, :], in_=wo_d[:, 2:4, :])

            # ---- persistent K^T and V ----
            kt = big.tile([128, T], f32r, tag="kt")
            v_sb = [
                big.tile([128, 128], f32r, tag=f"v{i}", name=f"v{i}")
                for i in range(T // 128)
            ]
            qt_all = {}
            ctxn = {}
            attn_state = {}

            def rope_evict(out_ap, ps, j):
                c = cs_sb[0:64, j * 512 : (j + 1) * 512]
                s = cs_sb[64:128, j * 512 : (j + 1) * 512]
                qe, qo = ps[0:64, :], ps[64:128, :]
                t1 = rope.tile([64, 512], f32, tag="rt1", name="rt1")
                t2 = rope.tile([64, 512], f32, tag="rt2", name="rt2")
                nc.vector.tensor_mul(t1[:], qe, c)
                nc.vector.tensor_mul(t2[:], qo, s)
                nc.gpsimd.tensor_sub(out_ap[0:64, :], t1[:], t2[:])
                t3 = rope.tile([64, 512], f32, tag="rt1", name="rt3")
                t4 = rope.tile([64, 512], f32, tag="rt2", name="rt4")
                nc.vector.tensor_mul(t3[:], qe, s)
                nc.vector.tensor_mul(t4[:], qo, c)
                nc.gpsimd.tensor_add(out_ap[64:128, :], t3[:], t4[:])

            def emit_A(j):
                """QKV projections + rope + V transpose for t-block j."""
                jt = slice(j * 512, (j + 1) * 512)
                xts = xt_tiles[j]
                qt_sb = []
                for m in range(G + 2):
                    ps = ps2.tile([128, 512], f32, tag="qkv", name=f"qkvps{j}_{m}")
                    for k in range(KB):
                        xt_t, xk0 = xts[k]
                        nc.tensor.matmul(
                            ps[:],
                            wqkv_sb[:, k, m * 128 : (m + 1) * 128],
                            xt_t[:, k - xk0, :],
                            start=(k == 0),
                            stop=(k == KB - 1),
                        )
                    if m < G:
                        qt = work.tile([128, 512], f32r, tag=f"qt{m}", bufs=1,
                                       name=f"qt{j}_{m}")
                        rope_evict(qt[:], ps, j)
                        qt_sb.append(qt)
                    elif m == G:
                        rope_evict(kt[:, jt], ps, j)
                    else:
                        vt_sb = work.tile([128, 512], f32r, tag="vt", name=f"vt{j}")
                        nc.scalar.copy(vt_sb[:], ps[:])
                        vps = ps2.tile([128, 512], f32r, tag="qkv", name=f"vps{j}")
                        for i in range(4):
                            nc.tensor.transpose(
                                vps[:, i * 128 : (i + 1) * 128],
                                vt_sb[:, i * 128 : (i + 1) * 128],
                                ident,
                            )
                        for i in range(4):
                            nc.scalar.copy(
                                v_sb[4 * j + i][:], vps[:, i * 128 : (i + 1) * 128]
                            )
                qt_all[j] = qt_sb
                if j + 1 < NJ:
                    load_xt(j + 1)

            def emit_attn_pair(j, h0, h1):
                """Logits+exp+mask+P@V for (tq-block j, heads h0/h1),
                block-interleaved so the exp chain of one head hides under the
                other head's matmuls. Denominator accumulates on PE."""
                nk = 4 * (j + 1)
                st = {}
                for h in (h0, h1):
                    st[h] = (
                        psc.tile([128, 512], f32, tag="ctx", name=f"ctxps{j}_{h}"),
                        psy.tile([128, 512], f32, tag="y", name=f"dps{j}_{h}"),
                    )
                for i in range(nk):
                    for h in (h0, h1):
                        ctx_ps, d_ps = st[h]
                        s_ps = pss.tile([128, 512], f32, tag="s",
                                        name=f"sps{j}_{h}_{i}")
                        nc.tensor.matmul(
                            s_ps[:],
                            kt[:, i * 128 : (i + 1) * 128],
                            qt_all[j][h][:],
                            start=True,
                            stop=True,
                        )
                        p_sb = ppool.tile([128, 512], f32r, tag="p",
                                          name=f"p{j}_{h}_{i}")
                        nc.scalar.activation(p_sb[:], s_ps[:], EXP, scale=SCALE)
                        if i >= 4 * j:
                            # causal: keep where i*128 + x <= j*512 + y
                            nc.gpsimd.affine_select(
                                out=p_sb[:],
                                in_=p_sb[:],
                                compare_op=mybir.AluOpType.is_ge,
                                fill=0.0,
                                base=j * 512 - i * 128,
                                pattern=[[1, 512]],
                                channel_multiplier=-1,
                            )
                        nc.tensor.matmul(
                            ctx_ps[:],
                            v_sb[i][:],
                            p_sb[:],
                            start=(i == 0),
                            stop=(i == nk - 1),
                        )
                        nc.tensor.matmul(
                            d_ps[:],
                            ones,
                            p_sb[:],
                            start=(i == 0),
                            stop=(i == nk - 1),
                        )
                for h in (h0, h1):
                    attn_state[(j, h)] = st[h]

            def emit_D(j, h):
                """Denominator replicate-matmul + fast reciprocal + normalize
                (fused ctx PSUM eviction)."""
                ctx_ps, d_ps = attn_state.pop((j, h))
                rd = work.tile([128, 512], f32, tag="rd", bufs=1, name=f"rd{j}_{h}")
                nc.vector.reciprocal_approx_fast(out=rd[:], in_=d_ps[:])
                cn = work.tile([128, 512], f32r, tag=f"ctxn{h}", bufs=1,
                               name=f"cn{j}_{h}")
                nc.vector.tensor_mul(cn[:], ctx_ps[:], rd[:])
                ctxn[h] = cn

            def emit_C(j):
                """Out-projection for the 4 t-row-blocks of t-block j."""
                for it in range(4):
                    y_sb = youtp.tile([128, 2048], f32, tag="y", name=f"ysb{j}_{it}")
                    for n in range(4):
                        y_ps = psy.tile([128, 512], f32, tag="y",
                                        name=f"yps{j}_{it}_{n}")
                        for h in range(G):
                            nc.tensor.matmul(
                                y_ps[:],
                                ctxn[h][:, it * 128 : (it + 1) * 128],
                                wo_sb[:, h, n * 512 : (n + 1) * 512],
                                start=(h == 0),
                                stop=(h == G - 1),
                            )
                        if n % 2 == 0:
                            nc.scalar.copy(y_sb[:, n * 512 : (n + 1) * 512], y_ps[:])
                        else:
                            nc.vector.tensor_copy(
                                y_sb[:, n * 512 : (n + 1) * 512], y_ps[:]
                            )
                    nc.sync.dma_start(
                        out=y_d[j * 512 + it * 128 : j * 512 + (it + 1) * 128, :],
                        in_=y_sb[:],
                    )

            # ---- schedule: A(j+1) before C(j) to keep PE dense across the
            # softmax-chain latency; D(h) trails its head by one attn block ----
            emit_A(0)
            for j in range(NJ):
                emit_attn_pair(j, 0, 1)
                emit_D(j, 0)
                emit_D(j, 1)
                emit_attn_pair(j, 2, 3)
                emit_D(j, 2)
                if j + 1 < NJ:
                    emit_A(j + 1)
                emit_D(j, 3)
                emit_C(j)
    nc.compile()
    return nc


def _prep_inputs(x, Wq, Wk, Wv, Wo, cos, sin):
    """Per-core host-side sharding + k-tile-major layouts for batched DMA."""
    perm = np.concatenate([np.arange(0, HD, 2), np.arange(1, HD, 2)])
    Wq_h = Wq.reshape(D, H, HD)[:, :, perm]  # de-interleave rope pairs
    Wk_h = Wk.reshape(D, KVH, HD)[:, :, perm]
    Wv_h = Wv.reshape(D, KVH, HD)
    cs = np.concatenate([cos.T, sin.T], axis=0)  # (128, T)
    cs = np.ascontiguousarray(cs, dtype=np.float32)
    consts = np.concatenate(
        [np.eye(128, dtype=np.float32), np.ones((128, 128), np.float32)], axis=1
    )
    in_maps = []
    for c in range(N_CORES):
        b, g = divmod(c, KVH)
        # xT in [128, KB, T]: partition-major k-tiles of x^T
        xT = np.ascontiguousarray(
            x[b].T.reshape(KB, 128, T).transpose(1, 0, 2), dtype=np.float32
        )
        wq_c = Wq_h[:, G * g : G * (g + 1), :].reshape(D, G * HD)
        wqkv = np.concatenate([wq_c, Wk_h[:, g, :], Wv_h[:, g, :]], axis=1)  # (D, MW)
        wqkv = np.ascontiguousarray(
            wqkv.reshape(KB, 128, MW).transpose(1, 0, 2), dtype=np.float32
        )
        wo_c = Wo[G * HD * g : G * HD * (g + 1), :]  # (512, D)
        wo_c = np.ascontiguousarray(
            wo_c.reshape(G, 128, D).transpose(1, 0, 2), dtype=np.float32
        )
        in_maps.append(
            {"xT": xT, "wqkv": wqkv, "wo": wo_c, "cs": cs, "consts": consts}
        )
    return in_maps


def kernel(x, Wq, Wk, Wv, Wo, bo, cos, sin, _trace=False):
    from concourse.bass_utils import run_bass_kernel_spmd

    if "nc" not in _CACHE:
        _CACHE["nc"] = _build()
    nc = _CACHE["nc"]

    in_maps = _prep_inputs(
        np.asarray(x), np.asarray(Wq), np.asarray(Wk), np.asarray(Wv),
        np.asarray(Wo), np.asarray(cos), np.asarray(sin),
    )
    res = run_bass_kernel_spmd(nc, in_maps, list(range(N_CORES)), trace=_trace)
    _CACHE["last_result"] = res

    bo = np.asarray(bo, dtype=np.float32)
    out = np.empty((B, T, D), dtype=np.float32)
    for b in range(B):
        acc = res.results[4 * b]["y"].astype(np.float32)
        for g in range(1, KVH):
            acc = acc + res.results[4 * b + g]["y"]
        out[b] = acc + bo[None, :]
    return out


# revision 15
# speedup vs baseline: 1.2040x; 1.2040x over previous
"""GQA MultiHeadAttention (B=2, T=2048, D=2048, H=16, KVH=4, HD=128) on 8 trn2 cores.

Sharding: core c -> (batch b = c//4, kv-head g = c%4). Each core computes its
4 query heads' attention + out-projection partial; host sums the 4 partials
per batch (row-parallel Wo all-reduce done on host at unshard time).

Device layout trick: everything stays transposed ([hd, t]) through attention:
  QT/KT = (Wq'^T x^T) with RoPE pair-deinterleave baked into the weight column
  order, S^T = KT-block-as-lhsT @ QT, P = exp(S^T) masked by affine_select,
  ctxT accumulates V-as-lhsT @ P in PSUM, denominator = all-ones matmul of the
  running P-sum (replicated across partitions) -> fast reciprocal -> multiply
  fused with the ctx PSUM eviction. ctxT feeds the out-projection as lhsT
  directly. Matmuls run in float32r (1 cyc/row at N=512 vs 4 for fp32).

Perf notes (from NTFF traces): PE in-order queue means emission order is the
schedule; A(j+1) is emitted before C(j) so the exp/denominator chains of B(j)
hide under A's dense matmuls. Denominator accumulation is split between DVE
and GPSIMD. DMAs are batched via host-side k-tile-major layouts.
"""

import sys

sys.path.insert(0, "/opt/trn_rl_repo")

import numpy as np

B, T, D = 2, 2048, 2048
H, KVH, HD = 16, 4, 128
G = H // KVH  # query heads per kv head (= heads per core)
KB = D // 128  # 16 contraction k-tiles
NJ = T // 512  # 4 t-blocks of 512
MW = (G + 2) * HD  # 768 qkv projection width per core
SCALE = 1.0 / np.sqrt(HD)
N_CORES = 8

_CACHE = {}


def _build():
    import concourse.bass as bass  # noqa: F401
    import concourse.tile as tile
    from concourse import bacc, mybir

    f32 = mybir.dt.float32
    f32r = mybir.dt.float32r
    EXP = mybir.ActivationFunctionType.Exp

    nc = bacc.Bacc(None, target_bir_lowering=False, debug=False)
    xT_d = nc.dram_tensor("xT", [128, KB, T], f32r, kind="ExternalInput")
    wqkv_d = nc.dram_tensor("wqkv", [128, KB, MW], f32r, kind="ExternalInput")
    wo_d = nc.dram_tensor("wo", [128, G, D], f32r, kind="ExternalInput")
    cs_d = nc.dram_tensor("cs", [128, T], f32, kind="ExternalInput")
    consts_d = nc.dram_tensor("consts", [128, 256], f32r, kind="ExternalInput")
    y_d = nc.dram_tensor("y", [T, D], f32, kind="ExternalOutput")

    with tile.TileContext(nc) as tc:
        with (
            tc.tile_pool(name="const", bufs=1) as const,
            tc.tile_pool(name="wq", bufs=1) as wqp,
            tc.tile_pool(name="wop", bufs=1) as wop,
            tc.tile_pool(name="xt", bufs=6) as xtp,
            tc.tile_pool(name="big", bufs=1) as big,
            tc.tile_pool(name="work", bufs=2) as work,
            tc.tile_pool(name="pp", bufs=4) as ppool,
            tc.tile_pool(name="rope", bufs=2) as rope,
            tc.tile_pool(name="yout", bufs=2) as youtp,
            tc.tile_pool(name="ps2", bufs=2, space="PSUM") as ps2,
            tc.tile_pool(name="pss", bufs=2, space="PSUM") as pss,
            tc.tile_pool(name="psc", bufs=2, space="PSUM") as psc,
            tc.tile_pool(name="psy", bufs=2, space="PSUM") as psy,
        ):
            # ---- resident weights / constants ----
            # startup: wqkv on the scalar HWDGE queue, xt(0) on sync, the
            # non-critical loads on gpsimd SWDGE -> three parallel streams
            wqkv_sb = wqp.tile([128, KB, MW], f32r, tag="wqkv")
            xt_tiles = {}

            def load_xt(j, eng=None):
                # chunks of k-tiles for t-block j (finer first chunks on j=0)
                chunks = [4, 4, 4, 4]
                tiles, k0 = [], 0
                for ci, w in enumerate(chunks):
                    t_ = xtp.tile([128, w, 512], f32r, tag="xt", name=f"xt{j}_{ci}",
                                  padded_shape=[128, 4, 512])
                    (eng or nc.sync).dma_start(
                        out=t_[:],
                        in_=xT_d[:, k0 : k0 + w, j * 512 : (j + 1) * 512],
                    )
                    for _ in range(w):
                        tiles.append((t_, k0))
                    k0 += w
                xt_tiles[j] = tiles

            consts_sb = const.tile([128, 256], f32r, tag="consts")
            nc.gpsimd.dma_start(out=consts_sb[:], in_=consts_d[:, :])
            ident = consts_sb[:, 0:128]
            ones = consts_sb[:, 128:256]
            def load_wq_chunk(k0, w):
                nc.sync.dma_start(
                    out=wqkv_sb[:, k0 : k0 + w, :], in_=wqkv_d[:, k0 : k0 + w, :]
                )

            def load_xt_chunk(j, tiles, ci, k0, w):
                t_ = xtp.tile([128, w, 512], f32r, tag="xt", name=f"xt{j}_{ci}",
                              padded_shape=[128, 4, 512])
                nc.sync.dma_start(
                    out=t_[:], in_=xT_d[:, k0 : k0 + w, j * 512 : (j + 1) * 512]
                )
                for _ in range(w):
                    tiles.append((t_, k0))

            xt0_tiles = []
            for ci in range(4):
                load_wq_chunk(4 * ci, 4)
                load_xt_chunk(0, xt0_tiles, ci, 4 * ci, 4)
            xt_tiles[0] = xt0_tiles
            cs_sb = const.tile([128, T], f32, tag="cs")
            nc.sync.dma_start(out=cs_sb[:], in_=cs_d[:, :])
            wo_sb = wop.tile([128, G, D], f32r, tag="wo")
            nc.sync.dma_start(out=wo_sb[:, 0:2, :], in_=wo_d[:, 0:2, :])
            nc.sync.dma_start(out=wo_sb[:, 2:4, :], in_=wo_d[:, 2:4, :])

            # ---- persistent K^T and V ----
            kt = big.tile([128, T], f32r, tag="kt")
            v_sb = [
                big.tile([128, 128], f32r, tag=f"v{i}", name=f"v{i}")
                for i in range(T // 128)
            ]
            qt_all = {}
            ctxn = {}
            attn_state = {}

            def rope_evict(out_ap, ps, j):
                c = cs_sb[0:64, j * 512 : (j + 1) * 512]
                s = cs_sb[64:128, j * 512 : (j + 1) * 512]
                qe, qo = ps[0:64, :], ps[64:128, :]
                t1 = rope.tile([64, 512], f32, tag="rt1", name="rt1")
                t2 = rope.tile([64, 512], f32, tag="rt2", name="rt2")
                nc.vector.tensor_mul(t1[:], qe, c)
                nc.vector.tensor_mul(t2[:], qo, s)
                nc.gpsimd.tensor_sub(out_ap[0:64, :], t1[:], t2[:])
                t3 = rope.tile([64, 512], f32, tag="rt1", name="rt3")
                t4 = rope.tile([64, 512], f32, tag="rt2", name="rt4")
                nc.vector.tensor_mul(t3[:], qe, s)
                nc.vector.tensor_mul(t4[:], qo, c)
                nc.gpsimd.tensor_add(out_ap[64:128, :], t3[:], t4[:])

            def emit_A(j):
                """QKV projections + rope + V transpose for t-block j."""
                jt = slice(j * 512, (j + 1) * 512)
                xts = xt_tiles[j]
                qt_sb = []
                for m in range(G + 2):
                    ps = ps2.tile([128, 512], f32, tag="qkv", name=f"qkvps{j}_{m}")
                    for k in range(KB):
                        xt_t, xk0 = xts[k]
                        nc.tensor.matmul(
                            ps[:],
                            wqkv_sb[:, k, m * 128 : (m + 1) * 128],
                            xt_t[:, k - xk0, :],
                            start=(k == 0),
                            stop=(k == KB - 1),
                        )
                    if m < G:
                        qt = work.tile([128, 512], f32r, tag=f"qt{m}", bufs=1,
                                       name=f"qt{j}_{m}")
                        rope_evict(qt[:], ps, j)
                        qt_sb.append(qt)
                    elif m == G:
                        rope_evict(kt[:, jt], ps, j)
                    else:
                        vt_sb = work.tile([128, 512], f32r, tag="vt", name=f"vt{j}")
                        nc.scalar.copy(vt_sb[:], ps[:])
                        vps = ps2.tile([128, 512], f32r, tag="qkv", name=f"vps{j}")
                        for i in range(4):
                            nc.tensor.transpose(
                                vps[:, i * 128 : (i + 1) * 128],
                                vt_sb[:, i * 128 : (i + 1) * 128],
                                ident,
                            )
                        for i in range(4):
                            nc.scalar.copy(
                                v_sb[4 * j + i][:], vps[:, i * 128 : (i + 1) * 128]
                            )
                qt_all[j] = qt_sb
                if j + 1 < NJ:
                    load_xt(j + 1)

            def emit_attn_pair(j, h0, h1):
                """Logits+exp+mask+P@V for (tq-block j, heads h0/h1),
                block-interleaved so the exp chain of one head hides under the
                other head's matmuls. Denominator accumulates on PE."""
                nk = 4 * (j + 1)
                st = {}
                for h in (h0, h1):
                    st[h] = (
                        psc.tile([128, 512], f32, tag="ctx", name=f"ctxps{j}_{h}"),
                        psy.tile([128, 512], f32, tag="y", name=f"dps{j}_{h}"),
                    )
                pending = []

                def flush(pair):
                    for h, i, p_sb in pair:
                        ctx_ps, d_ps = st[h]
                        nc.tensor.matmul(
                            ctx_ps[:], v_sb[i][:], p_sb[:],
                            start=(i == 0), stop=(i == nk - 1),
                        )
                        nc.tensor.matmul(
                            d_ps[:], ones, p_sb[:],
                            start=(i == 0), stop=(i == nk - 1),
                        )

                for i in range(nk):
                    cur = []
                    for h in (h0, h1):
                        s_ps = pss.tile([128, 512], f32, tag="s",
                                        name=f"sps{j}_{h}_{i}")
                        nc.tensor.matmul(
                            s_ps[:],
                            kt[:, i * 128 : (i + 1) * 128],
                            qt_all[j][h][:],
                            start=True,
                            stop=True,
                        )
                        p_sb = ppool.tile([128, 512], f32r, tag="p",
                                          name=f"p{j}_{h}_{i}")
                        nc.scalar.activation(p_sb[:], s_ps[:], EXP, scale=SCALE)
                        if i >= 4 * j:
                            # causal: keep where i*128 + x <= j*512 + y
                            nc.gpsimd.affine_select(
                                out=p_sb[:],
                                in_=p_sb[:],
                                compare_op=mybir.AluOpType.is_ge,
                                fill=0.0,
                                base=j * 512 - i * 128,
                                pattern=[[1, 512]],
                                channel_multiplier=-1,
                            )
                        cur.append((h, i, p_sb))
                    if pending:
                        flush(pending)
                    pending = cur
                flush(pending)
                for h in (h0, h1):
                    attn_state[(j, h)] = st[h]

            def emit_D(j, h):
                """Denominator replicate-matmul + fast reciprocal + normalize
                (fused ctx PSUM eviction)."""
                ctx_ps, d_ps = attn_state.pop((j, h))
                rd = work.tile([128, 512], f32, tag="rd", bufs=1, name=f"rd{j}_{h}")
                nc.vector.reciprocal_approx_fast(out=rd[:], in_=d_ps[:])
                cn = work.tile([128, 512], f32r, tag=f"ctxn{h}", bufs=1,
                               name=f"cn{j}_{h}")
                nc.vector.tensor_mul(cn[:], ctx_ps[:], rd[:])
                ctxn[h] = cn

            def emit_C(j):
                """Out-projection for the 4 t-row-blocks of t-block j."""
                for it in range(4):
                    y_sb = youtp.tile([128, 2048], f32, tag="y", name=f"ysb{j}_{it}")
                    for n in range(4):
                        y_ps = psy.tile([128, 512], f32, tag="y",
                                        name=f"yps{j}_{it}_{n}")
                        for h in range(G):
                            nc.tensor.matmul(
                                y_ps[:],
                                ctxn[h][:, it * 128 : (it + 1) * 128],
                                wo_sb[:, h, n * 512 : (n + 1) * 512],
                                start=(h == 0),
                                stop=(h == G - 1),
                            )
                        if n % 2 == 0:
                            nc.scalar.copy(y_sb[:, n * 512 : (n + 1) * 512], y_ps[:])
                        else:
                            nc.vector.tensor_copy(
                                y_sb[:, n * 512 : (n + 1) * 512], y_ps[:]
                            )
                    nc.sync.dma_start(
                        out=y_d[j * 512 + it * 128 : j * 512 + (it + 1) * 128, :],
                        in_=y_sb[:],
                    )

            # ---- schedule: A(j+1) before C(j) to keep PE dense across the
            # softmax-chain latency; D(h) trails its head by one attn block ----
            emit_A(0)
            for j in range(NJ):
                emit_attn_pair(j, 0, 1)
                emit_D(j, 0)
                emit_D(j, 1)
                emit_attn_pair(j, 2, 3)
                emit_D(j, 2)
                if j + 1 < NJ:
                    emit_A(j + 1)
                emit_D(j, 3)
                emit_C(j)
    nc.compile()
    return nc


def _prep_inputs(x, Wq, Wk, Wv, Wo, cos, sin):
    """Per-core host-side sharding + k-tile-major layouts for batched DMA."""
    perm = np.concatenate([np.arange(0, HD, 2), np.arange(1, HD, 2)])
    Wq_h = Wq.reshape(D, H, HD)[:, :, perm]  # de-interleave rope pairs
    Wk_h = Wk.reshape(D, KVH, HD)[:, :, perm]
    Wv_h = Wv.reshape(D, KVH, HD)
    cs = np.concatenate([cos.T, sin.T], axis=0)  # (128, T)
    cs = np.ascontiguousarray(cs, dtype=np.float32)
    consts = np.concatenate(
        [np.eye(128, dtype=np.float32), np.ones((128, 128), np.float32)], axis=1
    )
    in_maps = []
    for c in range(N_CORES):
        b, g = divmod(c, KVH)
        # xT in [128, KB, T]: partition-major k-tiles of x^T
        xT = np.ascontiguousarray(
            x[b].T.reshape(KB, 128, T).transpose(1, 0, 2), dtype=np.float32
        )
        wq_c = Wq_h[:, G * g : G * (g + 1), :].reshape(D, G * HD)
        wqkv = np.concatenate([wq_c, Wk_h[:, g, :], Wv_h[:, g, :]], axis=1)  # (D, MW)
        wqkv = np.ascontiguousarray(
            wqkv.reshape(KB, 128, MW).transpose(1, 0, 2), dtype=np.float32
        )
        wo_c = Wo[G * HD * g : G * HD * (g + 1), :]  # (512, D)
        wo_c = np.ascontiguousarray(
            wo_c.reshape(G, 128, D).transpose(1, 0, 2), dtype=np.float32
        )
        in_maps.append(
            {"xT": xT, "wqkv": wqkv, "wo": wo_c, "cs": cs, "consts": consts}
        )
    return in_maps


def kernel(x, Wq, Wk, Wv, Wo, bo, cos, sin, _trace=False):
    from concourse.bass_utils import run_bass_kernel_spmd

    if "nc" not in _CACHE:
        _CACHE["nc"] = _build()
    nc = _CACHE["nc"]

    in_maps = _prep_inputs(
        np.asarray(x), np.asarray(Wq), np.asarray(Wk), np.asarray(Wv),
        np.asarray(Wo), np.asarray(cos), np.asarray(sin),
    )
    res = run_bass_kernel_spmd(nc, in_maps, list(range(N_CORES)), trace=_trace)
    _CACHE["last_result"] = res

    bo = np.asarray(bo, dtype=np.float32)
    out = np.empty((B, T, D), dtype=np.float32)
    for b in range(B):
        acc = res.results[4 * b]["y"].astype(np.float32)
        for g in range(1, KVH):
            acc = acc + res.results[4 * b + g]["y"]
        out[b] = acc + bo[None, :]
    return out
